# revision 1
# baseline (speedup 1.0000x reference)
"""CrossAttention Trainium2 kernel — 8-core batch+head-parallel sharding.

Problem (hardcoded): B=2, N=M=2048, D=1024, H=16 heads x 64 dim, fp32.
  kv = ctx @ Wkv ; q = x @ Wq ; dots = (q k^T) * s - (1-mask)*1e6 (per query row)
  out = softmax(dots) @ v ; return out @ Wout + bout

Sharding: core c -> batch b = c//4, head group g = c%4 (4 heads each).
Each core computes its 4 heads' attention and a partial (row-parallel Wout)
output [2048, 1024]; host sums the 4 partials per batch (the "all-reduce"),
adds bout, and undoes the query permutation.

Precision strategy (validated vs reference in fp64/numpy simulation):
  - The mask penalty is an additive per-row constant, so softmax(x - 1e6)
    == softmax(x) mathematically, BUT jax's fp32 x-1e6 quantizes x to a
    0.0625 grid; reproducing that exactly needs dots at ~fp32 accuracy.
  - fp32 matmuls run at 1/4 rate on TRN2; float32r (11-bit mantissa
    inputs, fp32 accumulate) runs at full rate. We recover fp32-quality
    dots with a hi/lo split: a@b ~= ar@br + ae@br + ar@be where
    xr = round_fp32r(x), xe = round_fp32r(x - xr).  Measured 1e-7 rel.
  - q/k projections and q.k dots use the 3-term split; v/attn@v/Wout use
    plain float32r (no grid amplification downstream). End-to-end absmax
    error vs reference ~1e-3 (dominated by unavoidable 0.0625-grid
    rounding flips from summation-order noise).

Queries are sorted mask=1-first on the host so the penalty handling is
three compile-time column regions (clean1 / mixed / clean0); the mixed
region (covering the n1 spread across batches) uses a per-column penalty
vector from DRAM.
"""

import numpy as np

import concourse.bass as bass
import concourse.mybir as mybir
import concourse.tile as tile
from concourse import bacc
from concourse.bass_utils import run_bass_kernel_spmd

F32 = mybir.dt.float32
F32R = mybir.dt.float32r
AF = mybir.ActivationFunctionType
OP = mybir.AluOpType

B, NQ, NM, D, H, DH = 2, 2048, 2048, 1024, 16, 64
SCALE = np.float32(DH ** -0.5)
NCORES = 8
HPC = H // (NCORES // B)  # heads per core = 4
DHC = HPC * DH            # 256 head dims per core
NJ, JW = 4, 512           # n (query) chunks
NI, IW = 16, 128          # m (key) chunks
NKC, KW = 8, 128          # D contraction chunks
MASK_NEG = np.float32(1.0e6)


def _r32r(a):
    """Round fp32 -> float32r grid (11-bit mantissa, round-half-up)."""
    u = np.ascontiguousarray(a, np.float32).view(np.uint32)
    u = (u + np.uint32(1 << 12)) & np.uint32(0xFFFFE000)
    return u.view(np.float32)


def _split(a):
    r = _r32r(a)
    e = _r32r((a - r).astype(np.float32))
    return r, e


def build_program(segments):
    """segments: list over j-chunks of lists of (col_lo, col_hi, kind) with
    kind in {'c1','mx','c0'}; cols local to the j-chunk."""
    nc = bacc.Bacc("TRN2", target_bir_lowering=False, debug=False)

    din = {}
    for nm, shp, dt in [
        ("xrT", [D, NQ], F32R), ("xeT", [D, NQ], F32R),
        ("crT", [D, NM], F32R), ("ceT", [D, NM], F32R),
        ("wqr", [D, DHC], F32R), ("wqe", [D, DHC], F32R),
        ("wkr", [D, DHC], F32R), ("wke", [D, DHC], F32R),
        ("wv", [D, DHC], F32R),
        ("wo2a", [2 * DH, D], F32R), ("wo2b", [2 * DH, D], F32R),
        ("pen", [1, NQ], F32), ("ones1", [1, NI * HPC], F32R),
    ]:
        din[nm] = nc.dram_tensor(nm, shp, dt, kind="ExternalInput")
    po = nc.dram_tensor("po", [NQ, D], F32, kind="ExternalOutput")
    srec = nc.dram_tensor("srec", [16, JW], F32)

    with tile.TileContext(nc) as tc:
        with (
            tc.tile_pool(name="persist", bufs=1) as qkpool,
            tc.tile_pool(name="avnp", bufs=1) as avpool,
        ):
            # ---- persistent tiles (live across phases) ----
            wo2a = qkpool.tile([2 * DH, D], F32R, tag="wo2a")
            wo2b = qkpool.tile([2 * DH, D], F32R, tag="wo2b")
            nc.sync.dma_start(wo2a[:], din["wo2a"][:])
            nc.sync.dma_start(wo2b[:], din["wo2b"][:])
            penb = qkpool.tile([IW, NQ], F32, tag="penb")  # pen bcast over parts
            nc.sync.dma_start(
                penb[:], bass.AP(tensor=din["pen"][:].tensor, offset=0,
                                 ap=[[0, IW], [1, NQ]]))
            bias1e6 = qkpool.tile([IW, 1], F32, tag="bias1e6")
            nc.vector.memset(bias1e6[:], float(MASK_NEG))

            # qT/kT splits: per pg (head pair) [128, NQ/NM]
            qrT, qeT, krT, keT = {}, {}, {}, {}
            for pg in range(2):
                qrT[pg] = qkpool.tile([2 * DH, NQ], F32R, tag=f"qrT{pg}", name=f"qrT{pg}")
                qeT[pg] = qkpool.tile([2 * DH, NQ], F32R, tag=f"qeT{pg}", name=f"qeT{pg}")
                krT[pg] = qkpool.tile([2 * DH, NM], F32R, tag=f"krT{pg}", name=f"krT{pg}")
                keT[pg] = qkpool.tile([2 * DH, NM], F32R, tag=f"keT{pg}", name=f"keT{pg}")
            # v (+ones col): 16 m-chunk groups [128, 16, 4, 65]
            v_s = qkpool.tile([IW, NI, HPC, DH + 1], F32R, tag="v_s")
            # avn stacks per j: [128, 512] x 2
            avn = {(st, j): avpool.tile([2 * DH, JW], F32R, tag=f"avn{st}_{j}", name=f"avn{st}_{j}")
                   for st in range(2) for j in range(NJ)}

            # ---- phase 1: projections ----
            with (
                tc.tile_pool(name="projw", bufs=1) as wpool,
                tc.tile_pool(name="stream", bufs=4) as spool,
                tc.tile_pool(name="psP", bufs=3, space="PSUM") as psP,
                tc.tile_pool(name="psVv", bufs=4, space="PSUM") as psV,
            ):
                w = {}
                for nm in ("wqr", "wqe", "wkr", "wke", "wv"):
                    w[nm] = wpool.tile([KW, NKC * DHC], F32R, tag=nm, name=nm)
                    for kc in range(NKC):
                        nc.sync.dma_start(
                            w[nm][:, kc * DHC:(kc + 1) * DHC],
                            din[nm][kc * KW:(kc + 1) * KW, :])
                # q^T / k^T: out[pg][dh2, n] = sum_D w[D, dh2] * xT[D, n]
                for (dst_r, dst_e, wr_, we_, srcr, srce, nn) in (
                    (qrT, qeT, w["wqr"], w["wqe"], "xrT", "xeT", NQ),
                    (krT, keT, w["wkr"], w["wke"], "crT", "ceT", NM),
                ):
                    for jj in range(nn // JW):
                        ps = {pg: psP.tile([2 * DH, JW], F32, tag="psP",
                                           name=f"psP{pg}") for pg in range(2)}
                        for kc in range(NKC):
                            tr = spool.tile([KW, JW], F32R, tag="str_r")
                            te = spool.tile([KW, JW], F32R, tag="str_e")
                            nc.sync.dma_start(
                                tr[:],
                                din[srcr][kc * KW:(kc + 1) * KW, jj * JW:(jj + 1) * JW])
                            nc.sync.dma_start(
                                te[:],
                                din[srce][kc * KW:(kc + 1) * KW, jj * JW:(jj + 1) * JW])
                            for pg in range(2):
                                wr_c = wr_[:, kc * DHC + pg * 2 * DH:
                                           kc * DHC + (pg + 1) * 2 * DH]
                                we_c = we_[:, kc * DHC + pg * 2 * DH:
                                           kc * DHC + (pg + 1) * 2 * DH]
                                nc.tensor.matmul(ps[pg][:], wr_c, tr[:],
                                                 start=(kc == 0), stop=False)
                                nc.tensor.matmul(ps[pg][:], wr_c, te[:],
                                                 start=False, stop=False)
                                nc.tensor.matmul(ps[pg][:], we_c, tr[:],
                                                 start=False, stop=(kc == NKC - 1))
                        for pg in range(2):
                            # split evac: r = round(ps) on ACT; e = ps - r on DVE
                            sl = (slice(None), slice(jj * JW, (jj + 1) * JW))
                            nc.scalar.copy(dst_r[pg][sl], ps[pg][:])
                            nc.vector.tensor_tensor(
                                out=dst_e[pg][sl], in0=ps[pg][:],
                                in1=dst_r[pg][sl].bitcast(F32), op=OP.subtract)
                # v: out[m-chunk 128, h, 0:64] = sum_D ctxT[D, m]^T wv[D, :]
                for tj in range(NM // JW):
                    psv = {tt: psV.tile([IW, HPC * DH], F32, tag="psVv",
                                        name=f"psv{tt}") for tt in range(4)}
                    for kc in range(NKC):
                        trc = spool.tile([KW, JW], F32R, tag="str_r")
                        nc.sync.dma_start(
                            trc[:],
                            din["crT"][kc * KW:(kc + 1) * KW, tj * JW:(tj + 1) * JW])
                        for tt in range(4):
                            nc.tensor.matmul(
                                psv[tt][:], trc[:, tt * IW:(tt + 1) * IW],
                                w["wv"][:, kc * DHC:(kc + 1) * DHC],
                                start=(kc == 0), stop=(kc == NKC - 1))
                    for tt in range(4):
                        t = tj * 4 + tt
                        nc.scalar.copy(
                            v_s[:, t, :, 0:DH],
                            psv[tt][:].rearrange("p (h d) -> p h d", h=HPC))
                nc.sync.dma_start(
                    v_s[:, :, :, DH:DH + 1],
                    bass.AP(tensor=din["ones1"][:].tensor, offset=0,
                            ap=[[0, IW], [1, NI * HPC]]))

            # ---- phase 2: attention per (head-pair, j, head, i) ----
            with (
                tc.tile_pool(name="etile", bufs=4) as epool,
                tc.tile_pool(name="small", bufs=3) as smpool,
                tc.tile_pool(name="psD", bufs=3, space="PSUM") as psD,
                tc.tile_pool(name="psAV", bufs=4, space="PSUM") as psAV,
            ):
                for pg in range(2):
                    for j in range(NJ):
                        jsl = slice(j * JW, (j + 1) * JW)
                        av_ps = {}
                        for hh in range(2):
                            h = 2 * pg + hh
                            rsl = slice(hh * DH, (hh + 1) * DH)
                            av_ps[hh] = psAV.tile([DH + 1, JW], F32, tag="psAV", name=f"avps{hh}")
                            for i in range(NI):
                                pd = psD.tile([IW, JW], F32, tag="psD")
                                nc.tensor.matmul(pd[:], krT[pg][rsl, i * IW:(i + 1) * IW],
                                                 qrT[pg][rsl, jsl], start=True, stop=False)
                                nc.tensor.matmul(pd[:], krT[pg][rsl, i * IW:(i + 1) * IW],
                                                 qeT[pg][rsl, jsl], start=False, stop=False)
                                nc.tensor.matmul(pd[:], keT[pg][rsl, i * IW:(i + 1) * IW],
                                                 qrT[pg][rsl, jsl], start=False, stop=True)
                                # evac -> e tile (fp32r), per column segment
                                et = epool.tile([IW, JW], F32R, tag="e")
                                for (lo, hi, kind) in segments[j]:
                                    esl = (slice(None), slice(lo, hi))
                                    if kind == "c1":
                                        nc.scalar.activation(et[esl], pd[esl], AF.Exp)
                                    elif kind == "c0":
                                        tt = smpool.tile([IW, JW], F32, tag="t0")
                                        nc.vector.tensor_scalar(
                                            out=tt[esl], in0=pd[esl],
                                            scalar1=-float(MASK_NEG), scalar2=None,
                                            op0=OP.add)
                                        nc.scalar.activation(et[esl], tt[esl], AF.Exp,
                                                             bias=bias1e6[:])
                                    else:  # mx
                                        psl = (slice(None),
                                               slice(j * JW + lo, j * JW + hi))
                                        tt = smpool.tile([IW, JW], F32, tag="t0")
                                        nc.vector.tensor_tensor(
                                            out=tt[esl], in0=pd[esl], in1=penb[psl],
                                            op=OP.subtract)
                                        tt2 = smpool.tile([IW, JW], F32, tag="t1")
                                        nc.vector.tensor_tensor(
                                            out=tt2[esl], in0=tt[esl], in1=penb[psl],
                                            op=OP.add)
                                        nc.scalar.activation(et[esl], tt2[esl], AF.Exp)
                                # av accumulate: [65, JW] += v_ones[i,h]^T . e
                                nc.tensor.matmul(av_ps[hh][:], v_s[:, i, h, :], et[:],
                                                 start=(i == 0), stop=(i == NI - 1))
                        for hh in range(2):
                            # S = row 64; broadcast 1/S to 64 partitions via DRAM
                            r = pg * 8 + j * 2 + hh
                            srow = smpool.tile([1, JW], F32, tag="srow")
                            nc.vector.tensor_copy(srow[:], av_ps[hh][DH:DH + 1, :])
                            nc.sync.dma_start(srec[r:r + 1, :], srow[:])
                            sb = smpool.tile([DH, JW], F32, tag="sb")
                            nc.sync.dma_start(
                                sb[:], bass.AP(tensor=srec[:].tensor,
                                               offset=r * JW,
                                               ap=[[0, DH], [1, JW]]))
                            rec = smpool.tile([DH, JW], F32, tag="rec")
                            nc.vector.reciprocal(rec[:], sb[:])
                            nc.vector.tensor_tensor(
                                out=avn[(pg, j)][hh * DH:(hh + 1) * DH, :],
                                in0=av_ps[hh][0:DH, :], in1=rec[:], op=OP.mult)

            # ---- phase 3: out projection ----
            with (
                tc.tile_pool(name="outp", bufs=2) as opool,
                tc.tile_pool(name="psO", bufs=2, space="PSUM") as psO,
            ):
                for j in range(NJ):
                    for tt_ in range(4):
                        t = j * 4 + tt_
                        tsl = slice(tt_ * IW, (tt_ + 1) * IW)
                        ob = opool.tile([IW, D], F32, tag="ob")
                        for fc in range(2):
                            fsl = slice(fc * JW, (fc + 1) * JW)
                            pso = psO.tile([IW, JW], F32, tag="psO")
                            nc.tensor.matmul(pso[:], avn[(0, j)][:, tsl],
                                             wo2a[:, fsl], start=True, stop=False)
                            nc.tensor.matmul(pso[:], avn[(1, j)][:, tsl],
                                             wo2b[:, fsl], start=False, stop=True)
                            nc.scalar.copy(ob[:, fsl], pso[:])
                        nc.sync.dma_start(po[t * IW:(t + 1) * IW, :], ob[:])

    nc.compile()
    return nc


_CACHE = {}


def _segments_from_counts(n1s):
    """Compute per-j column segments from per-batch mask=1 counts."""
    lo = (min(n1s) // IW) * IW          # below lo: all mask1
    hi = -(-max(n1s) // IW) * IW        # above hi: all mask0
    segs = []
    for j in range(NJ):
        a, b_ = j * JW, (j + 1) * JW
        s = []
        c1e = min(max(lo, a), b_)
        mxe = min(max(hi, a), b_)
        if c1e > a:
            s.append((0, c1e - a, "c1"))
        if mxe > c1e:
            s.append((c1e - a, mxe - a, "mx"))
        if b_ > mxe:
            s.append((mxe - a, b_ - a, "c0"))
        segs.append(s)
    return tuple(tuple(x) for x in segs)


def kernel(x, context, mask, Wq, Wkv, Wout, bout):
    x = np.asarray(x, np.float32)
    context = np.asarray(context, np.float32)
    mask = np.asarray(mask)
    Wq = np.asarray(Wq, np.float32)
    Wkv = np.asarray(Wkv, np.float32)
    Wout = np.asarray(Wout, np.float32)
    bout = np.asarray(bout, np.float32)

    # Host-side shard prep.  Sort queries mask=1-first per batch.
    perms, n1s, pens = [], [], []
    for b in range(B):
        mb = (mask[b] != 0)
        perm = np.argsort(~mb, kind="stable")  # mask1 rows first
        perms.append(perm)
        n1s.append(int(mb.sum()))
        pens.append(((~mb[perm]).astype(np.float32) * MASK_NEG)[None, :])
    segs = _segments_from_counts(n1s)

    if segs not in _CACHE:
        _CACHE[segs] = build_program(segs)
    nc = _CACHE[segs]

    Wq_s = (Wq * SCALE).astype(np.float32)
    in_maps = []
    for c in range(NCORES):
        b, g = c // (NCORES // B), c % (NCORES // B)
        hsl = slice(g * DHC, (g + 1) * DHC)
        xT = np.ascontiguousarray(x[b][perms[b]].T)
        cT = np.ascontiguousarray(context[b].T)
        xr, xe = _split(xT)
        cr, ce = _split(cT)
        wqr, wqe = _split(np.ascontiguousarray(Wq_s[:, hsl]))
        wkr, wke = _split(np.ascontiguousarray(Wkv[:, hsl]))
        wv = _r32r(np.ascontiguousarray(Wkv[:, D + g * DHC: D + (g + 1) * DHC]))
        woc = Wout[hsl, :]
        in_maps.append({
            "xrT": xr, "xeT": xe, "crT": cr, "ceT": ce,
            "wqr": wqr, "wqe": wqe, "wkr": wkr, "wke": wke, "wv": wv,
            "wo2a": _r32r(np.ascontiguousarray(woc[0:2 * DH, :])),
            "wo2b": _r32r(np.ascontiguousarray(woc[2 * DH:4 * DH, :])),
            "pen": np.ascontiguousarray(pens[b]),
            "ones1": np.ones((1, NI * HPC), np.float32),
        })

    res = run_bass_kernel_spmd(nc, in_maps, core_ids=list(range(NCORES)))
    kernel.last_results = res

    out = np.empty((B, NQ, D), np.float32)
    for b in range(B):
        acc = res.results[b * 4]["po"].astype(np.float32).copy()
        for c in range(b * 4 + 1, b * 4 + 4):
            acc += res.results[c]["po"]
        unperm = np.empty_like(acc)
        unperm[perms[b]] = acc + bout[None, :]
        out[b] = unperm
    return out



# revision 13
# speedup vs baseline: 2.4712x; 2.4712x over previous
"""CrossAttention Trainium2 kernel — 8-core batch+head-parallel sharding.

Problem (hardcoded): B=2, N=M=2048, D=1024, H=16 heads x 64 dim, fp32.
  kv = ctx @ Wkv ; q = x @ Wq ; dots = (q k^T) * s - (1-mask)*1e6 (per query row)
  out = softmax(dots) @ v ; return out @ Wout + bout

Sharding: core c -> batch b = c//4, head group g = c%4 (4 heads each).
Each core computes its 4 heads' attention and a partial (row-parallel Wout)
output [2048, 1024]; host sums the 4 partials per batch, adds bout.

Numerics: the mask penalty is an additive per-query-row constant, so
softmax(x - 1e6) == softmax(x) mathematically; the reference output only
feels it through fp32 quantization (x - 1e6 rounds x to a 0.0625 grid).
We skip the mask entirely and run everything in plain float32r (11-bit
mantissa inputs, fp32 accumulate). Measured end-to-end l2 rel-err vs the
fp32 reference: ~8e-3 (dominated by the reference's own grid-quantization
noise on masked rows), comfortably inside the 2e-2 gate.

Per-core engine budget (est): PE ~140us (projections 41, dots 28 with
head-pair row-packing, attn@v 56 at M=65, out-proj 14), ACT ~135us
(16.8M exps in [128,1024] instructions), DVE/Pool light.
"""

import numpy as np

import concourse.bass as bass
import concourse.mybir as mybir
import concourse.tile as tile
from concourse import bacc
from concourse.bass_utils import run_bass_kernel_spmd

F32 = mybir.dt.float32
F32R = mybir.dt.float32r
AF = mybir.ActivationFunctionType
OP = mybir.AluOpType

B, NQ, NM, D, H, DH = 2, 2048, 2048, 1024, 16, 64
SCALE = np.float32(DH ** -0.5)
NCORES = 8
HPC = H // (NCORES // B)  # heads per core = 4
DHC = HPC * DH            # 256 head dims per core
NJ, JW = 4, 512           # n (query) chunks
NI, IW = 16, 128          # m (key) chunks
NKC, KW = 8, 128          # D contraction chunks


def _r32r(a):
    """Round fp32 -> float32r grid (11-bit mantissa, round-half-up)."""
    u = np.ascontiguousarray(a, np.float32).view(np.uint32)
    u = (u + np.uint32(1 << 12)) & np.uint32(0xFFFFE000)
    return u.view(np.float32)


def build_program(debug=False):
    nc = bacc.Bacc("TRN2", target_bir_lowering=False, debug=False)

    din = {}
    for nm, shp, dt in [
        ("xT", [D, NQ], F32R), ("cT", [D, NM], F32R),
        ("wq", [D, DHC], F32R), ("wk", [D, DHC], F32R), ("wv", [D, DHC], F32R),
        ("wo2", [2 * DH, 2 * D], F32R),
        ("ones1", [1, NI * HPC], F32R),
    ]:
        din[nm] = nc.dram_tensor(nm, shp, dt, kind="ExternalInput")
    po = nc.dram_tensor("po", [NQ, D], F32, kind="ExternalOutput")
    dbg = {}
    if debug:
        for nm, shp, dt in [
            ("d_qT0", [2 * DH, NQ], F32R), ("d_kT0", [2 * DH, NM], F32R),
            ("d_vs", [IW, NI * HPC * (DH + 1)], F32R),
            ("d_et00", [IW, 2 * JW], F32R),
            ("d_rbc00", [DH, JW], F32),
            ("d_avn00", [2 * DH, JW], F32R), ("d_avn10", [2 * DH, JW], F32R),
        ]:
            dbg[nm] = nc.dram_tensor(nm, shp, dt, kind="ExternalOutput")

    with tile.TileContext(nc) as tc:
        with tc.tile_pool(name="persist", bufs=1) as pp:
            # ---- persistent SBUF tiles ----
            wq_sb = pp.tile([KW, NKC * DHC], F32R, tag="wq_sb")
            wk_sb = pp.tile([KW, NKC * DHC], F32R, tag="wk_sb")
            wv_sb = pp.tile([KW, NKC * DHC], F32R, tag="wv_sb")
            wo2_sb = pp.tile([2 * DH, 2 * D], F32R, tag="wo2_sb")
            for kc in range(NKC):
                for w_sb, w_dr in ((wq_sb, "wq"), (wk_sb, "wk"), (wv_sb, "wv")):
                    nc.sync.dma_start(
                        w_sb[:, kc * DHC:(kc + 1) * DHC],
                        din[w_dr][kc * KW:(kc + 1) * KW, :])
            nc.sync.dma_start(wo2_sb[:], din["wo2"][:])

            qT = {pg: pp.tile([2 * DH, NQ], F32R, tag=f"qT{pg}", name=f"qT{pg}")
                  for pg in range(2)}
            kT = {pg: pp.tile([2 * DH, NM], F32R, tag=f"kT{pg}", name=f"kT{pg}")
                  for pg in range(2)}
            # v (+ones col): [m 128, i 16, h 4, d 65]
            v_s = pp.tile([IW, NI, HPC, DH + 1], F32R, tag="v_s")
            nc.sync.dma_start(
                v_s[:, :, :, DH:DH + 1],
                bass.AP(tensor=din["ones1"][:].tensor, offset=0,
                        ap=[[0, IW], [1, NI * HPC]]))
            avn = {(pg, j): pp.tile([2 * DH, JW], F32R, tag=f"avn{pg}_{j}",
                                    name=f"avn{pg}_{j}")
                   for pg in range(2) for j in range(NJ)}

            # ---- phase A: k and v projections (stream cT once) ----
            with (
                tc.tile_pool(name="streamA", bufs=6) as sA,
                tc.tile_pool(name="psK", bufs=2, space="PSUM") as psKp,
                tc.tile_pool(name="psV", bufs=4, space="PSUM") as psVp,
            ):
                for jj in range(NJ):
                    jsl = slice(jj * JW, (jj + 1) * JW)
                    psK = psKp.tile([2 * DH, 2 * JW], F32, tag="psK")
                    # one accumulation group per PSUM bank: 4 one-bank tiles
                    psV = {tt: psVp.tile([IW, DHC], F32, tag="psv",
                                         name=f"psv{tt}") for tt in range(4)}
                    for kc in range(NKC):
                        t = sA.tile([KW, JW], F32R, tag="ct")
                        nc.sync.dma_start(
                            t[:], din["cT"][kc * KW:(kc + 1) * KW, jsl])
                        for pg in range(2):
                            nc.tensor.matmul(
                                psK[:, pg * JW:(pg + 1) * JW],
                                wk_sb[:, kc * DHC + pg * 2 * DH:
                                      kc * DHC + (pg + 1) * 2 * DH],
                                t[:], start=(kc == 0), stop=(kc == NKC - 1))
                        for tt in range(4):
                            nc.tensor.matmul(
                                psV[tt][:],
                                t[:, tt * IW:(tt + 1) * IW],
                                wv_sb[:, kc * DHC:(kc + 1) * DHC],
                                start=(kc == 0), stop=(kc == NKC - 1))
                    for pg in range(2):
                        nc.scalar.copy(kT[pg][:, jsl], psK[:, pg * JW:(pg + 1) * JW])
                    for tt in range(4):
                        nc.scalar.copy(
                            v_s[:, jj * 4 + tt, :, 0:DH],
                            psV[tt][:].rearrange("p (h d) -> p h d", h=HPC))

            # ---- phase B: q proj (interleaved), attention, out proj ----
            with (
                tc.tile_pool(name="streamB", bufs=6) as sB,
                tc.tile_pool(name="etp", bufs=3) as ep,
                tc.tile_pool(name="smallB", bufs=2) as smp,
                tc.tile_pool(name="obp", bufs=2) as obp,
                tc.tile_pool(name="psD", bufs=2, space="PSUM") as pdp,
                tc.tile_pool(name="psAV", bufs=2, space="PSUM") as avp,
                tc.tile_pool(name="psFlex", bufs=2, space="PSUM") as fxp,
            ):
                def qproj(jj):
                    jsl = slice(jj * JW, (jj + 1) * JW)
                    ps = {pg: fxp.tile([2 * DH, JW], F32, tag="flex",
                                       name=f"psQ{pg}") for pg in range(2)}
                    for kc in range(NKC):
                        t = sB.tile([KW, JW], F32R, tag="xt")
                        nc.sync.dma_start(
                            t[:], din["xT"][kc * KW:(kc + 1) * KW, jsl])
                        for pg in range(2):
                            nc.tensor.matmul(
                                ps[pg][:],
                                wq_sb[:, kc * DHC + pg * 2 * DH:
                                      kc * DHC + (pg + 1) * 2 * DH],
                                t[:], start=(kc == 0), stop=(kc == NKC - 1))
                    for pg in range(2):
                        nc.vector.tensor_copy(qT[pg][:, jsl], ps[pg][:])

                def att_block(pg, j):
                    jsl = slice(j * JW, (j + 1) * JW)
                    av = {hh: avp.tile([DH + 1, JW], F32, tag="av",
                                       name=f"av{hh}") for hh in range(2)}
                    pend = []  # pipelined (pd, et, i) awaiting exp+av

                    def drain():
                        pd_, i_ = pend.pop(0)
                        et = ep.tile([IW, 2 * JW], F32R, tag="et")
                        nc.scalar.activation(et[:], pd_[:], AF.Exp)
                        if debug and pg == 0 and j == 0 and i_ == 0:
                            nc.sync.dma_start(dbg["d_et00"][:], et[:])
                        for hh in range(2):
                            nc.tensor.matmul(
                                av[hh][:], v_s[:, i_, 2 * pg + hh, :],
                                et[:, hh * JW:(hh + 1) * JW],
                                start=(i_ == 0), stop=(i_ == NI - 1))

                    for i in range(NI):
                        pd = pdp.tile([IW, 2 * JW], F32, tag="pd")
                        for hh in range(2):
                            hsl = slice(hh * DH, (hh + 1) * DH)
                            nc.tensor.matmul(
                                pd[:, hh * JW:(hh + 1) * JW],
                                kT[pg][hsl, i * IW:(i + 1) * IW],
                                qT[pg][hsl, jsl], start=True, stop=True)
                        pend.append((pd, i))
                        if len(pend) > 1:
                            drain()
                    drain()

                    for hh in range(2):
                        srow = smp.tile([1, JW], F32, tag="srow")
                        nc.vector.tensor_copy(srow[:], av[hh][DH:DH + 1, :])
                        rec = smp.tile([1, JW], F32, tag="rec")
                        nc.vector.reciprocal_approx_fast(rec[:], srow[:])
                        rbc = smp.tile([DH, JW], F32, tag="rbc")
                        nc.gpsimd.partition_broadcast(rbc[:], rec[:], channels=DH)
                        if debug and pg == 0 and j == 0 and hh == 0:
                            nc.sync.dma_start(dbg["d_rbc00"][:], rbc[:])
                        nc.vector.tensor_tensor(
                            out=avn[(pg, j)][hh * DH:(hh + 1) * DH, :],
                            in0=av[hh][0:DH, :], in1=rbc[:], op=OP.mult)

                def outproj(j):
                    for t4 in range(4):
                        tsl = slice(t4 * IW, (t4 + 1) * IW)
                        ob = obp.tile([IW, D], F32, tag="ob")
                        for fc in range(2):
                            fsl = slice(fc * JW, (fc + 1) * JW)
                            pso = fxp.tile([IW, JW], F32, tag="flex", name="pso")
                            nc.tensor.matmul(pso[:], avn[(0, j)][:, tsl],
                                             wo2_sb[:, fc * JW:(fc + 1) * JW],
                                             start=True, stop=False)
                            nc.tensor.matmul(pso[:], avn[(1, j)][:, tsl],
                                             wo2_sb[:, D + fc * JW:D + (fc + 1) * JW],
                                             start=False, stop=True)
                            nc.vector.tensor_copy(ob[:, fsl], pso[:])
                        nc.sync.dma_start(
                            po[j * JW + t4 * IW: j * JW + (t4 + 1) * IW, :],
                            ob[:])

                qproj(0)
                for j in range(NJ):
                    att_block(0, j)
                    if j < NJ - 1:
                        qproj(j + 1)
                    att_block(1, j)
                    outproj(j)

                if debug:
                    nc.sync.dma_start(dbg["d_qT0"][:], qT[0][:])
                    nc.sync.dma_start(dbg["d_kT0"][:], kT[0][:])
                    nc.sync.dma_start(
                        dbg["d_vs"][:],
                        v_s[:].rearrange("p a b c -> p (a b c)"))
                    nc.sync.dma_start(dbg["d_avn00"][:], avn[(0, 0)][:])
                    nc.sync.dma_start(dbg["d_avn10"][:], avn[(1, 0)][:])

    nc.compile()
    return nc


_CACHE = {}


def kernel(x, context, mask, Wq, Wkv, Wout, bout):
    x = np.asarray(x, np.float32)
    context = np.asarray(context, np.float32)
    Wq = np.asarray(Wq, np.float32)
    Wkv = np.asarray(Wkv, np.float32)
    Wout = np.asarray(Wout, np.float32)
    bout = np.asarray(bout, np.float32)

    if "nc" not in _CACHE:
        _CACHE["nc"] = build_program()
    nc = _CACHE["nc"]

    Wq_s = (Wq * SCALE).astype(np.float32)
    xT = [_r32r(x[b].T) for b in range(B)]
    cT = [_r32r(context[b].T) for b in range(B)]
    ones1 = np.ones((1, NI * HPC), np.float32)

    in_maps = []
    for c in range(NCORES):
        b, g = c // (NCORES // B), c % (NCORES // B)
        hsl = slice(g * DHC, (g + 1) * DHC)
        woc = Wout[hsl, :]
        wo2 = np.concatenate([woc[0:2 * DH, :], woc[2 * DH:4 * DH, :]], axis=1)
        in_maps.append({
            "xT": xT[b], "cT": cT[b],
            "wq": _r32r(np.ascontiguousarray(Wq_s[:, hsl])),
            "wk": _r32r(np.ascontiguousarray(Wkv[:, hsl])),
            "wv": _r32r(np.ascontiguousarray(
                Wkv[:, D + g * DHC: D + (g + 1) * DHC])),
            "wo2": _r32r(np.ascontiguousarray(wo2)),
            "ones1": ones1,
        })

    res = run_bass_kernel_spmd(nc, in_maps, core_ids=list(range(NCORES)))
    kernel.last_results = res

    out = np.empty((B, NQ, D), np.float32)
    for b in range(B):
        acc = res.results[b * 4]["po"].astype(np.float32).copy()
        for c in range(b * 4 + 1, b * 4 + 4):
            acc += res.results[c]["po"]
        out[b] = acc + bout[None, :]
    return out


# revision 22
# speedup vs baseline: 3.1720x; 1.2836x over previous
"""CrossAttention Trainium2 kernel — 8-core batch+head-parallel sharding.

Problem (hardcoded): B=2, N=M=2048, D=1024, H=16 heads x 64 dim, fp32.
  kv = ctx @ Wkv ; q = x @ Wq ; dots = (q k^T) * s - (1-mask)*1e6 (per query row)
  out = softmax(dots) @ v ; return out @ Wout + bout

Sharding: core c -> batch b = c//4, head group g = c%4 (4 heads each).
Each core computes its 4 heads' attention and a partial (row-parallel Wout)
output [2048, 1024]; host sums the 4 partials per batch, adds bout.

Numerics: the mask penalty is an additive per-query-row constant, so
softmax(x - 1e6) == softmax(x) mathematically; the reference output only
feels it through fp32 quantization (x - 1e6 rounds x to a 0.0625 grid).
We skip the mask entirely and run everything in plain float32r (11-bit
mantissa inputs, fp32 accumulate). Measured end-to-end l2 rel-err vs the
fp32 reference: ~8e-3, inside the 2e-2 gate.

Schedule: phase A streams cT and computes kT + v (PE), evacs on ACT.
Attention blocks (head-pair pg x query-chunk j) run a software pipeline:
dots(i) issued at iter i (row-packed head pair, one [128,1024] 2-bank
PSUM tile), exp(i) on ACT at iter i+1 ([128,1024] single instruction),
attn@v(i) at iter i+2 so the PE never waits on ACT semaphores. q
projections for chunk j+1 and the out-projection of chunk j-1 are
emitted as fillers inside the blocks to absorb boundary latency.
"""

import numpy as np

import concourse.bass as bass
import concourse.mybir as mybir
import concourse.tile as tile
from concourse import bacc
from concourse.bass_utils import run_bass_kernel_spmd

F32 = mybir.dt.float32
F32R = mybir.dt.float32r
AF = mybir.ActivationFunctionType
OP = mybir.AluOpType

B, NQ, NM, D, H, DH = 2, 2048, 2048, 1024, 16, 64
SCALE = np.float32(DH ** -0.5)
NCORES = 8
HPC = H // (NCORES // B)  # heads per core = 4
DHC = HPC * DH            # 256 head dims per core
NJ, JW = 4, 512           # n (query) chunks
NI, IW = 16, 128          # m (key) chunks
NKC, KW = 8, 128          # D contraction chunks


def _r32r(a):
    """Round fp32 -> float32r grid (11-bit mantissa, round-half-up)."""
    u = np.ascontiguousarray(a, np.float32).view(np.uint32)
    u = (u + np.uint32(1 << 12)) & np.uint32(0xFFFFE000)
    return u.view(np.float32)


def build_program(debug=False):
    nc = bacc.Bacc("TRN2", target_bir_lowering=False, debug=False)

    din = {}
    for nm, shp, dt in [
        ("xT", [D, NQ], F32R), ("cT", [D, NM], F32R),
        ("wq", [D, DHC], F32R), ("wk", [D, DHC], F32R), ("wv", [D, DHC], F32R),
        ("wo2", [2 * DH, 2 * D], F32R),
    ]:
        din[nm] = nc.dram_tensor(nm, shp, dt, kind="ExternalInput")
    po = nc.dram_tensor("po", [NQ, D], F32, kind="ExternalOutput")
    dbg = {}
    if debug:
        for nm, shp, dt in [
            ("d_qT0", [2 * DH, NQ], F32R), ("d_kT0", [2 * DH, NM], F32R),
            ("d_vs", [IW, NI * HPC * (DH + 1)], F32R),
            ("d_et00", [IW, 2 * JW], F32R),
            ("d_rbc00", [DH, JW], F32),
            ("d_avn00", [2 * DH, JW], F32R), ("d_avn10", [2 * DH, JW], F32R),
        ]:
            dbg[nm] = nc.dram_tensor(nm, shp, dt, kind="ExternalOutput")

    def dma_chunk4(eng, dst_tile, src_name, kc0, col0):
        """DMA [128, 4, JW]: partitions p <- src row (kc0+kcl)*128+p,
        free (kcl, c) <- col col0+c. Single 3-level-AP transfer."""
        src = din[src_name]
        ncols = src.shape[1]
        eng.dma_start(
            dst_tile[:],
            bass.AP(tensor=src[:].tensor,
                    offset=kc0 * KW * ncols + col0,
                    ap=[[ncols, KW], [KW * ncols, 4], [1, JW]]))

    with tile.TileContext(nc) as tc:
        with tc.tile_pool(name="persist", bufs=1) as pp:
            # ---- persistent SBUF tiles ----
            wq_sb = pp.tile([KW, NKC, DHC], F32R, tag="wq_sb")
            wk_sb = pp.tile([KW, NKC, DHC], F32R, tag="wk_sb")
            wv_sb = pp.tile([KW, NKC, DHC], F32R, tag="wv_sb")
            wo2_sb = pp.tile([2 * DH, 2 * D], F32R, tag="wo2_sb")
            # weights on the gpsimd SWDGE queue; cT streams own the SP queue
            for w_sb, w_dr in ((wk_sb, "wk"), (wv_sb, "wv"), (wq_sb, "wq")):
                nc.gpsimd.dma_start(
                    w_sb[:],
                    bass.AP(tensor=din[w_dr][:].tensor, offset=0,
                            ap=[[DHC, KW], [KW * DHC, NKC], [1, DHC]]))
            nc.gpsimd.dma_start(wo2_sb[:], din["wo2"][:])

            qT = {pg: pp.tile([2 * DH, NQ], F32R, tag=f"qT{pg}", name=f"qT{pg}")
                  for pg in range(2)}
            kT = {pg: pp.tile([2 * DH, NM], F32R, tag=f"kT{pg}", name=f"kT{pg}")
                  for pg in range(2)}
            # v (+ones col): [m 128, i 16, h 4, d 65]
            v_s = pp.tile([IW, NI, HPC, DH + 1], F32R, tag="v_s")
            nc.vector.memset(v_s[:, :, :, DH:DH + 1].bitcast(F32), 1.0)
            avn = {(pg, j): pp.tile([2 * DH, JW], F32R, tag=f"avn{pg}_{j}",
                                    name=f"avn{pg}_{j}")
                   for pg in range(2) for j in range(NJ)}

            # ---- phase A: k and v projections (stream cT once) ----
            with (
                tc.tile_pool(name="streamA", bufs=4) as sA,
                tc.tile_pool(name="psK", bufs=2, space="PSUM") as psKp,
                tc.tile_pool(name="psV", bufs=4, space="PSUM") as psVp,
            ):
                for jj in range(NJ):
                    jsl = slice(jj * JW, (jj + 1) * JW)
                    psK = psKp.tile([2 * DH, 2 * JW], F32, tag="psK")
                    psV = {tt: psVp.tile([IW, DHC], F32, tag="psv",
                                         name=f"psv{tt}") for tt in range(4)}
                    th = {}
                    for half in range(2):
                        th[half] = sA.tile([KW, 4, JW], F32R, tag="ct4",
                                           name=f"ct4_{half}")
                        dma_chunk4(nc.sync, th[half], "cT", half * 4, jj * JW)
                    for kc in range(NKC):
                        t = th[kc // 4][:, kc % 4, :]
                        for pg in range(2):
                            nc.tensor.matmul(
                                psK[:, pg * JW:(pg + 1) * JW],
                                wk_sb[:, kc, pg * 2 * DH:(pg + 1) * 2 * DH],
                                t, start=(kc == 0), stop=(kc == NKC - 1))
                        for tt in range(4):
                            nc.tensor.matmul(
                                psV[tt][:],
                                t[:, tt * IW:(tt + 1) * IW],
                                wv_sb[:, kc, :],
                                start=(kc == 0), stop=(kc == NKC - 1))
                    for pg in range(2):
                        nc.scalar.copy(kT[pg][:, jsl], psK[:, pg * JW:(pg + 1) * JW])
                    for tt in range(4):
                        nc.scalar.copy(
                            v_s[:, jj * 4 + tt, :, 0:DH],
                            psV[tt][:].rearrange("p (h d) -> p h d", h=HPC))

            # ---- phase B: attention + interleaved q proj and out proj ----
            with (
                tc.tile_pool(name="streamB", bufs=4) as sB,
                tc.tile_pool(name="etp", bufs=4) as ep,
                tc.tile_pool(name="smallB", bufs=2) as smp,
                tc.tile_pool(name="obp", bufs=2) as obp,
                tc.tile_pool(name="psD", bufs=2, space="PSUM") as pdp,
                tc.tile_pool(name="psAV", bufs=2, space="PSUM") as avp,
                tc.tile_pool(name="psFlex", bufs=2, space="PSUM") as fxp,
            ):
                xtiles = {}

                def xdma(jj):
                    # x stream rides the ACT HWDGE queue (parallel to cT on SP)
                    xtiles[jj] = {}
                    for half in range(2):
                        t = sB.tile([KW, 4, JW], F32R, tag="xt4",
                                    name=f"xt4_{half}")
                        dma_chunk4(nc.scalar, t, "xT", half * 4, jj * JW)
                        xtiles[jj][half] = t

                def qproj_gen(jj):
                    """Yields after each chunk of PE work (2 MMs)."""
                    jsl = slice(jj * JW, (jj + 1) * JW)
                    ps = {pg: fxp.tile([2 * DH, JW], F32, tag="flex",
                                       name=f"psQ{pg}") for pg in range(2)}
                    for kc in range(NKC):
                        t = xtiles[jj][kc // 4][:, kc % 4, :]
                        for pg in range(2):
                            nc.tensor.matmul(
                                ps[pg][:],
                                wq_sb[:, kc, pg * 2 * DH:(pg + 1) * 2 * DH],
                                t, start=(kc == 0), stop=(kc == NKC - 1))
                        yield
                    del xtiles[jj]
                    for pg in range(2):
                        nc.vector.tensor_copy(qT[pg][:, jsl], ps[pg][:])
                    yield

                def outproj_gen(j):
                    """Yields after each (t4, fc) MM pair + evac."""
                    for t4 in range(4):
                        tsl = slice(t4 * IW, (t4 + 1) * IW)
                        ob = obp.tile([IW, D], F32, tag="ob")
                        for fc in range(2):
                            fsl = slice(fc * JW, (fc + 1) * JW)
                            pso = fxp.tile([IW, JW], F32, tag="flex", name="pso")
                            nc.tensor.matmul(pso[:], avn[(0, j)][:, tsl],
                                             wo2_sb[:, fc * JW:(fc + 1) * JW],
                                             start=True, stop=False)
                            nc.tensor.matmul(pso[:], avn[(1, j)][:, tsl],
                                             wo2_sb[:, D + fc * JW:D + (fc + 1) * JW],
                                             start=False, stop=True)
                            nc.vector.tensor_copy(ob[:, fsl], pso[:])
                            yield
                        nc.gpsimd.dma_start(
                            po[j * JW + t4 * IW: j * JW + (t4 + 1) * IW, :],
                            ob[:])

                def att_block(pg, j, fillers=()):
                    jsl = slice(j * JW, (j + 1) * JW)
                    av = {hh: avp.tile([DH + 1, JW], F32, tag="av",
                                       name=f"av{hh}") for hh in range(2)}
                    pend_exp = []   # (pd, i) awaiting exp
                    pend_av = []    # (et, i) awaiting attn@v
                    fill = list(fillers)

                    def do_exp():
                        pd_, i_ = pend_exp.pop(0)
                        et = ep.tile([IW, 2 * JW], F32R, tag="et")
                        nc.scalar.activation(et[:], pd_[:], AF.Exp)
                        if debug and pg == 0 and j == 0 and i_ == 0:
                            nc.sync.dma_start(dbg["d_et00"][:], et[:])
                        pend_av.append((et, i_))

                    def do_av():
                        et, i_ = pend_av.pop(0)
                        for hh in range(2):
                            nc.tensor.matmul(
                                av[hh][:], v_s[:, i_, 2 * pg + hh, :],
                                et[:, hh * JW:(hh + 1) * JW],
                                start=(i_ == 0), stop=(i_ == NI - 1))

                    for i in range(NI):
                        pd = pdp.tile([IW, 2 * JW], F32, tag="pd")
                        for hh in range(2):
                            hsl = slice(hh * DH, (hh + 1) * DH)
                            nc.tensor.matmul(
                                pd[:, hh * JW:(hh + 1) * JW],
                                kT[pg][hsl, i * IW:(i + 1) * IW],
                                qT[pg][hsl, jsl], start=True, stop=True)
                        pend_exp.append((pd, i))
                        if len(pend_exp) > 1:
                            do_exp()
                        if len(pend_av) > 1:
                            do_av()
                        if 2 <= i and fill:
                            for _ in range(2):
                                try:
                                    next(fill[0])
                                except StopIteration:
                                    fill.pop(0)
                                    if not fill:
                                        break
                    while pend_exp:
                        do_exp()
                    while pend_av:
                        do_av()
                    for g_ in fill:
                        for _ in g_:
                            pass

                    for hh in range(2):
                        srow = smp.tile([1, JW], F32, tag="srow")
                        nc.vector.tensor_copy(srow[:], av[hh][DH:DH + 1, :])
                        rec = smp.tile([1, JW], F32, tag="rec")
                        nc.vector.reciprocal_approx_fast(rec[:], srow[:])
                        rbc = smp.tile([DH, JW], F32, tag="rbc")
                        nc.gpsimd.partition_broadcast(rbc[:], rec[:], channels=DH)
                        if debug and pg == 0 and j == 0 and hh == 0:
                            nc.sync.dma_start(dbg["d_rbc00"][:], rbc[:])
                        nc.vector.tensor_tensor(
                            out=avn[(pg, j)][hh * DH:(hh + 1) * DH, :],
                            in0=av[hh][0:DH, :], in1=rbc[:], op=OP.mult)

                # schedule: prefetch x(0),x(1); qproj(0) inline; blocks with
                # qproj(j+1) + outproj(j-1) as fillers inside att(0, j)
                xdma(0)
                xdma(1)
                for _ in qproj_gen(0):
                    pass
                for j in range(NJ):
                    f0 = [qproj_gen(j + 1)] if j + 1 < NJ else []
                    if j > 0:
                        f0.append(outproj_gen(j - 1))
                    att_block(0, j, fillers=f0)
                    if j + 2 < NJ:
                        xdma(j + 2)
                    att_block(1, j)
                for _ in outproj_gen(NJ - 1):
                    pass

                if debug:
                    nc.sync.dma_start(dbg["d_qT0"][:], qT[0][:])
                    nc.sync.dma_start(dbg["d_kT0"][:], kT[0][:])
                    nc.sync.dma_start(
                        dbg["d_vs"][:],
                        v_s[:].rearrange("p a b c -> p (a b c)"))
                    nc.sync.dma_start(dbg["d_avn00"][:], avn[(0, 0)][:])
                    nc.sync.dma_start(dbg["d_avn10"][:], avn[(1, 0)][:])

    nc.compile()
    return nc


_CACHE = {}


def kernel(x, context, mask, Wq, Wkv, Wout, bout):
    x = np.asarray(x, np.float32)
    context = np.asarray(context, np.float32)
    Wq = np.asarray(Wq, np.float32)
    Wkv = np.asarray(Wkv, np.float32)
    Wout = np.asarray(Wout, np.float32)
    bout = np.asarray(bout, np.float32)

    if "nc" not in _CACHE:
        _CACHE["nc"] = build_program()
    nc = _CACHE["nc"]

    Wq_s = (Wq * SCALE).astype(np.float32)
    xT = [_r32r(x[b].T) for b in range(B)]
    cT = [_r32r(context[b].T) for b in range(B)]

    in_maps = []
    for c in range(NCORES):
        b, g = c // (NCORES // B), c % (NCORES // B)
        hsl = slice(g * DHC, (g + 1) * DHC)
        woc = Wout[hsl, :]
        wo2 = np.concatenate([woc[0:2 * DH, :], woc[2 * DH:4 * DH, :]], axis=1)
        in_maps.append({
            "xT": xT[b], "cT": cT[b],
            "wq": _r32r(np.ascontiguousarray(Wq_s[:, hsl])),
            "wk": _r32r(np.ascontiguousarray(Wkv[:, hsl])),
            "wv": _r32r(np.ascontiguousarray(
                Wkv[:, D + g * DHC: D + (g + 1) * DHC])),
            "wo2": _r32r(np.ascontiguousarray(wo2)),
        })

    res = run_bass_kernel_spmd(nc, in_maps, core_ids=list(range(NCORES)))
    kernel.last_results = res

    out = np.empty((B, NQ, D), np.float32)
    for b in range(B):
        acc = res.results[b * 4]["po"].astype(np.float32).copy()
        for c in range(b * 4 + 1, b * 4 + 4):
            acc += res.results[c]["po"]
        out[b] = acc + bout[None, :]
    return out
